# revision 21
# baseline (speedup 1.0000x reference)
"""Darknet 3x3 conv block (conv * mask + bias) on 8 TRN2 NeuronCores.

Problem: x[1,512,192,192] (*) w[512,512,3,3] stride1 pad1, then *mask + bias.

Strategy (v6): 1D Winograd F(4,3) along W, spatial shard over H.
  - Each core computes 24 output rows x all 512 F.
  - conv2d = sum_kh conv1d_W(x_row, w[kh]); the 1D conv uses Winograd
    F(4,3): 6 points per 4 outputs -> 2x fewer MACs than direct.
  - Host packs: x zero-padded, phase-split AND pre-scaled into 10 planes
    (d1..d5, 4d0, 4d1, 4d2, 5d2, 5d3) so the device input transform is
    14 plain tensor_tensor adds per (row-chunk, cc-pair) - no fused
    scalar ops (2x slower on DVE).  Flat per-(chunk, cc-pair) segments
    so every x DMA is one contiguous run.  Weights pre-transformed
    U[p] = G @ w.  Mask per-(group) contiguous, k-split.  Output bf16 in
    per-(fm,g) contiguous segments; host deinterleaves and upcasts.
  - Device: GEMM per (fm, group of 8 rows): two PSUM tiles (points 0-2 /
    3-5, 3 banks each, pool bufs=2), 36 matmuls each, lhsT =
    U[fm,p,cc,kh] [c128,f128], rhs = V[cc,p,rows,tiles] [128,384].
    ScalarE (only) copies PSUM->SBUF bf16; DVE does the factored A^T
    combine writing into the copy tile, mask-mul and bias-add (keeps
    ScalarE FIFO free of cross-engine stalls), one bf16 DMA per group.
  - g-outer / fm-inner group order: all 4 fm's reuse each V row band,
    so transforms only need 1/4 of PE rate.  U halves alternate between
    the ACT and SP DMA rings so each fm's weights land just in time.
"""

import sys

for _p in ("/opt/trn_rl_repo",):
    if _p not in sys.path:
        sys.path.insert(0, _p)

import numpy as np
import ml_dtypes

N_CORES = 8
C = 512
F = 512
H = 192
W = 192
HC = H // N_CORES          # output rows per core = 24
CC = C // 128              # c chunks = 4
FM = F // 128              # f chunks = 4
P = 6                      # Winograd F(4,3) points
NPH = 10                   # pre-scaled x phase planes
KH = 3                     # vertical taps (direct)
T = W // 4                 # Winograd tiles per row = 48
RG = 8                     # output rows per PSUM group
NG = HC // RG              # groups per fm = 3
NPX = RG * T               # matmul free size = 384
NWARM = 40                 # PE warmup matmuls while first DMAs land
XR = HC + 2                # x slab rows = 26

# plane indices: d1 d2 d3 d4 d5 4d0 4d1 4d2 5d2 5d3
PD1, PD2, PD3, PD4, PD5, P4D0, P4D1, P4D2, P5D2, P5D3 = range(NPH)

# row chunks for the input transform (independent: 1D transform).
# chunk 0 covers exactly what group g=0 needs (V rows 0..9).
CHUNKS = [(0, 10), (10, 8), (18, 8)]
XSEG = [2 * NPH * nr * T for _, nr in CHUNKS]
XOFF = {}
_off = 0
for _ci in range(len(CHUNKS)):
    for _pr in range(2):
        XOFF[(_ci, _pr)] = _off
        _off += XSEG[_ci]
XTOT = _off

_CACHE = {}


def _build():
    import concourse.bacc as bacc
    import concourse.mybir as mybir
    from concourse.tile import TileContext

    BF = mybir.dt.bfloat16
    F32 = mybir.dt.float32

    nc = bacc.Bacc(trn_type="TRN2", num_devices=N_CORES)
    x_sh = nc.dram_tensor("x_sh", [128, XTOT], BF, kind="ExternalInput")
    u_sh = nc.dram_tensor("u_sh", [128, FM, 2, 3, CC, KH, 128], BF,
                          kind="ExternalInput")
    mk_sh = nc.dram_tensor("mk_sh", [128, NG, 4, RG, T], BF,
                           kind="ExternalInput")
    b_sh = nc.dram_tensor("b_sh", [128, FM], F32, kind="ExternalInput")
    y_sh = nc.dram_tensor("y_sh", [FM, NG, 128, 4, RG, T], BF,
                          kind="ExternalOutput")

    with TileContext(nc) as tc:
        with (
            tc.tile_pool(name="const", bufs=1) as cpool,
            tc.tile_pool(name="xin", bufs=2) as xpool,
            tc.tile_pool(name="vscr", bufs=1) as spool,
            tc.tile_pool(name="psum", bufs=7, space="PSUM") as ppool,
            tc.tile_pool(name="pwarm", bufs=1, space="PSUM") as wpool,
            tc.tile_pool(name="cpy", bufs=2) as cpool2,
            tc.tile_pool(name="tmp", bufs=2) as tpool,
        ):
            # PE warmup while the first DMAs land (HAM pre-warm + head fill)
            scratch = cpool.tile([128, NPX], BF)
            nc.vector.memset(scratch[:], 0.0)
            dps = wpool.tile([128, 512], F32, name="dps", tag="pw")
            for _ in range(NWARM):
                nc.tensor.matmul(dps[:, :NPX], scratch[:, :128], scratch[:],
                                 start=True, stop=True)

            ut = cpool.tile([128, FM, 2, 3, CC, KH, 128], BF)
            mt = cpool.tile([128, NG, 4, RG, T], BF)
            bt = cpool.tile([128, FM], F32)
            vt = cpool.tile([128, CC, P, XR, T], BF)

            # U halves alternate rings: A-halves (points 0-2) + mask/bias
            # on the ACT ring; B-halves ride the SP ring interleaved with
            # the x chunks, each just ahead of first use.
            nc.scalar.dma_start(out=ut[:, 0, 0], in_=u_sh[:, 0, 0])
            nc.scalar.dma_start(out=mt[:], in_=mk_sh[:])
            nc.scalar.dma_start(out=bt[:], in_=b_sh[:])
            for fm in range(1, FM):
                nc.scalar.dma_start(out=ut[:, fm, 0], in_=u_sh[:, fm, 0])

            xts = {}

            def xdma(ci, pr, engine=None):
                xt = xpool.tile([128, 2, NPH, CHUNKS[ci][1], T], BF,
                                name=f"x_{ci}_{pr}", tag="xt")
                seg = x_sh[:, XOFF[(ci, pr)]:XOFF[(ci, pr)] + XSEG[ci]]
                (engine or nc.sync).dma_start(
                    out=xt[:],
                    in_=seg.rearrange("p (c j r t) -> p c j r t",
                                      c=2, j=NPH, t=T))
                xts[(ci, pr)] = xt

            def xwarm(ci, pr, n):
                # garbage matmuls on the freshly-landed x tile: keeps the
                # PE HAM clock warm through the head DMA bubble
                xf = xts[(ci, pr)][:].rearrange("p c j r t -> p (c j r t)")
                for i in range(n):
                    nc.tensor.matmul(dps[:, :NPX], scratch[:, :128],
                                     xf[:, i * NPX:(i + 1) * NPX],
                                     start=True, stop=True)

            # x chunk0 pair1 rides the ACT ring (in parallel with pair0 on
            # SP); the rest go SP in first-use order, U B-halves between.
            xdma(0, 0)
            xdma(0, 1)
            nc.sync.dma_start(out=ut[:, 0, 1], in_=u_sh[:, 0, 1])
            xdma(1, 0)
            xdma(1, 1)
            nc.sync.dma_start(out=ut[:, 1, 1], in_=u_sh[:, 1, 1])
            nc.sync.dma_start(out=ut[:, 2, 1], in_=u_sh[:, 2, 1])
            xdma(2, 0)
            xdma(2, 1)
            nc.sync.dma_start(out=ut[:, 3, 1], in_=u_sh[:, 3, 1])

            def transform(ci, pr):
                r0, nr = CHUNKS[ci]
                xt = xts[(ci, pr)]
                st = spool.tile([128, 2, nr, T], BF, name=f"s_{ci}_{pr}",
                                tag="st", bufs=1)
                s2 = spool.tile([128, 2, nr, T], BF, name=f"s2_{ci}_{pr}",
                                tag="st2", bufs=1)
                v = vt[:, 2 * pr:2 * pr + 2, :, r0:r0 + nr]

                def xp(j):
                    return xt[:, :, j]

                e = nc.vector
                # A-half points first so the A matmuls can start early.
                # V0 = (4d0 - 5d2) + d4
                e.tensor_sub(st[:], xp(P4D0), xp(P5D2))
                e.tensor_add(v[:, :, 0], st[:], xp(PD4))
                # m1 = d4 - 4d2;  m2 = d3 - 4d1;  V1 = m1+m2, V2 = m1-m2
                e.tensor_sub(st[:], xp(PD4), xp(P4D2))
                e.tensor_sub(s2[:], xp(PD3), xp(P4D1))
                e.tensor_add(v[:, :, 1], st[:], s2[:])
                e.tensor_sub(v[:, :, 2], st[:], s2[:])
                # m3 = d4 - d2;  m4 = d3 - d1
                e.tensor_sub(st[:], xp(PD4), xp(PD2))
                e.tensor_sub(s2[:], xp(PD3), xp(PD1))
                # V3 = (m3 + m4) + m4;  V4 = (m3 - m4) - m4
                e.tensor_add(v[:, :, 3], st[:], s2[:])
                e.tensor_add(v[:, :, 3], v[:, :, 3], s2[:])
                e.tensor_sub(v[:, :, 4], st[:], s2[:])
                e.tensor_sub(v[:, :, 4], v[:, :, 4], s2[:])
                # V5 = (4d1 - 5d3) + d5
                e.tensor_sub(st[:], xp(P4D1), xp(P5D3))
                e.tensor_add(v[:, :, 5], st[:], xp(PD5))

            def group(fm, g):
                # point-major: each Winograd point accumulates 12 matmuls
                # into its own 1-bank PSUM tile (bufs=7 -> PE can run ~7
                # points ahead of the ScalarE drains), then ScalarE
                # copies it to SBUF bf16.
                cp = cpool2.tile([128, 6, RG, T], BF, name=f"cp_{fm}_{g}",
                                 tag="cp")
                for p in range(P):
                    pt = ppool.tile([128, 512], F32, name=f"ps_{fm}_{g}_{p}",
                                    tag="ps")
                    for cc in range(CC):
                        for kh in range(KH):
                            rhs = vt[:, cc, p,
                                     RG * g + kh:RG * g + kh + RG, :]
                            nc.tensor.matmul(
                                pt[:, :NPX], ut[:, fm, p // 3, p % 3, cc, kh],
                                rhs,
                                start=(cc == 0 and kh == 0),
                                stop=(cc == CC - 1 and kh == KH - 1),
                            )
                    nc.scalar.copy(
                        cp[:, p],
                        pt[:, :NPX].rearrange("p (r t) -> p r t", t=T))

                m0, m1, m2 = cp[:, 0], cp[:, 1], cp[:, 2]
                m3, m4, m5 = cp[:, 3], cp[:, 4], cp[:, 5]
                tmp = tpool.tile([128, 6, RG, T], BF, name=f"tm_{fm}_{g}",
                                 tag="tm")
                s, dd, t0 = tmp[:, 0], tmp[:, 1], tmp[:, 2]
                pp, q, y3b = tmp[:, 3], tmp[:, 4], tmp[:, 5]
                MULT = mybir.AluOpType.mult
                ADD = mybir.AluOpType.add
                nc.vector.tensor_add(s, m1, m2)
                nc.vector.tensor_sub(dd, m1, m2)
                nc.vector.tensor_add(t0, m0, s)
                nc.vector.tensor_add(pp, m3, m4)
                nc.vector.tensor_sub(q, m3, m4)
                nc.vector.scalar_tensor_tensor(y3b, q, 8.0, m5, MULT, ADD)

                # combine writes into the cp slots (m0..m3 are dead by
                # their write time); mask + bias follow on DVE.
                nc.vector.tensor_add(cp[:, 0], t0, pp)
                nc.vector.scalar_tensor_tensor(cp[:, 1], q, 2.0, dd, MULT, ADD)
                nc.vector.scalar_tensor_tensor(cp[:, 2], pp, 4.0, s, MULT, ADD)
                nc.vector.tensor_add(cp[:, 3], y3b, dd)

                nc.vector.tensor_mul(cp[:, 0:4], cp[:, 0:4], mt[:, g])
                nc.vector.tensor_scalar_add(cp[:, 0:4], cp[:, 0:4],
                                            bt[:, fm:fm + 1])
                nc.sync.dma_start(out=y_sh[fm, g], in_=cp[:, 0:4])

            # g-outer, fm-inner: all four fm's reuse each V row band, so
            # the DVE transform pace only has to keep up with 1/4 of the
            # PE rate.  Emission interleaves transforms and drains so
            # both PSUM and cp slots recycle on time.
            transform(0, 0)
            transform(0, 1)
            transform(1, 0)
            transform(1, 1)
            group(0, 0)
            group(1, 0)
            group(2, 0)
            group(3, 0)
            transform(2, 0)
            transform(2, 1)
            group(0, 1)
            group(1, 1)
            group(2, 1)
            group(3, 1)
            for fm in range(FM):
                group(fm, 2)

    nc.compile()
    return nc


def _wino_mats():
    BT = np.array([
        [4, 0, -5, 0, 1, 0],
        [0, -4, -4, 1, 1, 0],
        [0, 4, -4, -1, 1, 0],
        [0, -2, -1, 2, 1, 0],
        [0, 2, -1, -2, 1, 0],
        [0, 4, 0, -5, 0, 1]], dtype=np.float64)
    G = np.array([
        [1 / 4, 0, 0],
        [-1 / 6, -1 / 6, -1 / 6],
        [-1 / 6, 1 / 6, -1 / 6],
        [1 / 24, 1 / 12, 1 / 6],
        [1 / 24, -1 / 12, 1 / 6],
        [0, 0, 1]], dtype=np.float64)
    AT = np.array([
        [1, 1, 1, 1, 1, 0],
        [0, 1, -1, 2, -2, 0],
        [0, 1, 1, 4, 4, 0],
        [0, 1, -1, 8, -8, 1]], dtype=np.float64)
    return BT, G, AT


def _pack(x, w, b, mask):
    x = np.asarray(x, dtype=np.float32)
    w = np.asarray(w, dtype=np.float32)
    b = np.asarray(b, dtype=np.float32)
    mask = np.asarray(mask)

    xp = np.zeros((C, H + 2, W + 2), dtype=np.float32)
    xp[:, 1:-1, 1:-1] = x[0]
    # phase split: ph[j][c, r, t] = xp[c, r, 4t + j],  j = 0..5
    ph = [xp[:, :, j:j + 4 * (T - 1) + 1:4] for j in range(6)]
    # pre-scaled planes: d1 d2 d3 d4 d5 4d0 4d1 4d2 5d2 5d3
    planes = [ph[1], ph[2], ph[3], ph[4], ph[5],
              4 * ph[0], 4 * ph[1], 4 * ph[2], 5 * ph[2], 5 * ph[3]]
    xd = np.stack(planes, axis=1).astype(ml_dtypes.bfloat16)
    # xd: [C, NPH, H+2, T]

    _, G, _ = _wino_mats()
    # U[p, f, c, kh] = sum_j G[p, j] * w[f, c, kh, j]
    u = np.einsum("pj,fckj->pfck", G, w.astype(np.float64)).astype(np.float32)
    # -> [c_local, fm, half, pj, cc, kh, f_local]
    u = u.reshape(2, 3, FM, 128, CC, 128, KH)
    u = u.transpose(5, 2, 0, 1, 4, 6, 3)
    u = np.ascontiguousarray(u).astype(ml_dtypes.bfloat16)

    b_re = np.ascontiguousarray(b.reshape(FM, 128).T)

    in_maps = []
    for k in range(N_CORES):
        xs = xd[:, :, HC * k:HC * k + XR, :].reshape(CC, 128, NPH, XR, T)
        segs = []
        for ci, (r0, nr) in enumerate(CHUNKS):
            for pr in range(2):
                seg = xs[2 * pr:2 * pr + 2, :, :, r0:r0 + nr, :]
                segs.append(np.ascontiguousarray(
                    seg.transpose(1, 0, 2, 3, 4)).reshape(128, -1))
        xflat = np.concatenate(segs, axis=1)
        assert xflat.shape == (128, XTOT)
        # per-core mask rows: global rows HC*k .. HC*k+HC-1
        mcore = mask[HC * k:HC * k + HC].reshape(NG, RG, T, 4)
        mcore = np.ascontiguousarray(
            mcore.transpose(0, 3, 1, 2)).astype(ml_dtypes.bfloat16)
        in_maps.append({
            "x_sh": np.ascontiguousarray(xflat),
            "u_sh": u,
            "mk_sh": np.ascontiguousarray(
                np.broadcast_to(mcore[None], (128, NG, 4, RG, T))),
            "b_sh": b_re,
        })
    return in_maps


def _unpack(results):
    slabs = []
    for k in range(N_CORES):
        ys = results[k]["y_sh"]                 # [FM, NG, 128, 4, RG, T] bf16
        ys = ys.transpose(0, 2, 1, 4, 5, 3)     # [FM, 128, NG, RG, T, 4]
        slabs.append(ys.reshape(F, HC, W))
    out = np.concatenate(slabs, axis=1)         # [512, 192, 192]
    return out[None].astype(np.float32)


def _run(inputs, **run_kwargs):
    from concourse.bass_utils import run_bass_kernel_spmd

    if "nc" not in _CACHE:
        _CACHE["nc"] = _build()
    nc = _CACHE["nc"]
    in_maps = _pack(inputs["x"], inputs["w"], inputs["b"], inputs["mask"])
    res = run_bass_kernel_spmd(nc, in_maps, core_ids=list(range(N_CORES)), **run_kwargs)
    return _unpack(res.results), res


def kernel(**inputs):
    out, _ = _run(inputs)
    return out
